# revision 7
# baseline (speedup 1.0000x reference)
"""Multi-head attention TRN2 Bass kernel, head-sharded across 8 NeuronCores.

Problem: S=2048, E=1024, H=16 heads, dk=dv=64, fp32.
    Q = x @ Wq.T ; K = x @ Wk.T ; V = x @ Wv.T   (per-head slices)
    A_h = softmax(Q_h K_h^T / 8) V_h
    out = concat_h(A_h) @ Wo.T

Sharding: tensor-parallel over heads. Core i owns heads (2i, 2i+1), computes
its heads' Q/K/V projections over the full sequence, attention, and a partial
output projection against the matching 128-column slice of Wo. The 8 partial
[2048,1024] outputs are summed on the host (the gather/unshard step).

On-chip layout (everything "transposed" so no PE transposes of big tensors
are needed and softmax normalization rides the AV matmul as a ones column):
    xT   [E, S]   E on partitions (8 chunks of 128)
    QT/KT/VT [128, S]: rows 0-63 head A, 64-127 head B  (dk on partitions)
    scores^T chunks [sk=128, sq] via zero-padded KT as lhsT
    exp on ACT (scale=1/8 fused), output bf16
    AV:  A^T[dv+1, sq] = [V_chunk | ones]^T-style lhsT, rhs = exp(scores^T)
         row 64 accumulates the softmax denominator for free
    outproj: lhsT = normalized A1^T [128, sq], rhs = WoT slice [128, E]
"""

import numpy as np

import concourse.bass as bass
import concourse.mybir as mybir
import concourse.tile as tile
from concourse import bacc
from concourse.bass_utils import run_bass_kernel_spmd
from concourse.masks import make_identity

S, E, H, DK, DV = 2048, 1024, 16, 64, 64
NCORES = 8
HPC = H // NCORES          # heads per core = 2
CSL = HPC * DV             # concat-dim columns per core = 128
P = 128
NE = E // P                # 8 contraction chunks for projections
SQB = 512                  # sequence block (moving-operand width)
NSQB = S // SQB            # 4
NCH = S // P               # 16 sk chunks of 128
F32 = mybir.dt.float32
F32R = mybir.dt.float32r
BF16 = mybir.dt.bfloat16
SCALE = 1.0 / np.sqrt(DK).astype(np.float32)  # 1/8

EXP = mybir.ActivationFunctionType.Exp
MULT = mybir.AluOpType.mult

_cache = {}
last_results = None  # BassKernelResults of the most recent run (for test.py)
TRACE = False


def _build_nc():
    nc = bacc.Bacc("TRN2", target_bir_lowering=False, debug=False)

    xT = nc.dram_tensor("xT", [E, S], F32R, kind="ExternalInput")
    wqT = nc.dram_tensor("wqT", [E, CSL], F32R, kind="ExternalInput")
    wkT = nc.dram_tensor("wkT", [E, CSL], F32R, kind="ExternalInput")
    wvT = nc.dram_tensor("wvT", [E, CSL], F32R, kind="ExternalInput")
    woT = nc.dram_tensor("woT", [CSL, E], F32R, kind="ExternalInput")
    y = nc.dram_tensor("y", [S, E], F32, kind="ExternalOutput")

    xT_r = xT.ap().rearrange("(n p) s -> p n s", p=P)
    w_r = {
        "q": wqT.ap().rearrange("(n p) m -> p n m", p=P),
        "k": wkT.ap().rearrange("(n p) m -> p n m", p=P),
        "v": wvT.ap().rearrange("(n p) m -> p n m", p=P),
    }
    y_ap = y.ap()

    with tile.TileContext(nc) as tc:
        with tc.tile_pool(name="persist", bufs=1) as persist:
            # Persistent SBUF tensors
            qt = persist.tile([P, S], F32R)          # QT, both heads stacked
            kpadA = persist.tile([P, S], F32R)       # KT head A rows 0-63, zeros below
            kpadB = persist.tile([P, S], F32R)       # KT head B rows 64-127, zeros above
            vaug = [
                persist.tile([P, NCH, DV + 1], BF16, name=f"vaug{h}", tag=f"vaug{h}")
                for h in range(HPC)
            ]
            wosb = persist.tile([P, E], F32R)
            ident = persist.tile([P, P], F32)

            make_identity(nc, ident[:])
            for h in range(HPC):
                nc.gpsimd.memset(vaug[h][:, :, DV : DV + 1], 1.0)

            nc.sync.dma_start(wosb[:], woT.ap())

            # ---- Phase B: QKV projections (QT/KT/VT = W_slice^T.T @ x^T) ----
            with tc.tile_pool(name="xw", bufs=1) as xw, \
                 tc.tile_pool(name="vtp", bufs=1) as vtp, \
                 tc.tile_pool(name="proj_ps", bufs=2, space="PSUM") as proj_ps, \
                 tc.tile_pool(name="tp_ps", bufs=2, space="PSUM") as tp_ps:
                xsb = xw.tile([P, NE, S], F32R)
                for n in range(NE):
                    nc.sync.dma_start(xsb[:, n, :], xT_r[:, n, :])
                wsb = {}
                for m in ("q", "k", "v"):
                    wsb[m] = xw.tile(
                        [P, NE, CSL], F32R, name=f"w{m}sb", tag=f"w{m}"
                    )
                    nc.sync.dma_start(wsb[m][:], w_r[m][:])

                vt = vtp.tile([P, S], F32)  # VT stacked, then transposed away

                for m in ("q", "k", "v"):
                    for t in range(NSQB):
                        sl = slice(t * SQB, (t + 1) * SQB)
                        ps = proj_ps.tile([P, SQB], F32, tag="proj")
                        for n in range(NE):
                            nc.tensor.matmul(
                                ps[:], lhsT=wsb[m][:, n, :], rhs=xsb[:, n, sl],
                                start=(n == 0), stop=(n == NE - 1),
                            )
                        if m == "q":
                            nc.vector.tensor_copy(qt[:, sl], ps[:])
                        elif m == "k":
                            nc.vector.tensor_copy(kpadA[0:DK, sl], ps[0:DK, :])
                            nc.vector.tensor_copy(kpadB[DK:P, sl], ps[DK:P, :])
                            # zero pads (f32r-rounded writes; memset can't)
                            nc.vector.tensor_scalar_mul(
                                kpadA[DK:P, sl], ps[DK:P, :], 0.0
                            )
                            nc.vector.tensor_scalar_mul(
                                kpadB[0:DK, sl], ps[0:DK, :], 0.0
                            )
                        else:
                            nc.vector.tensor_copy(vt[:, sl], ps[:])

                # ---- Phase C: VT -> V chunks (PE transpose), cast to bf16 ----
                for h in range(HPC):
                    rows = slice(h * DK, h * DK + DK)
                    idsl = ident[rows, rows]
                    for c in range(NCH):
                        tp = tp_ps.tile([P, DV], F32, tag="tp")
                        nc.tensor.transpose(
                            tp[:], vt[rows, c * P : (c + 1) * P], idsl
                        )
                        nc.vector.tensor_copy(vaug[h][:, c, 0:DV], tp[:])

            # ---- Phase D: attention + output projection, per sq block ----
            with tc.tile_pool(name="sc_ps", bufs=3, space="PSUM") as sc_ps, \
                 tc.tile_pool(name="av_ps", bufs=2, space="PSUM") as av_ps, \
                 tc.tile_pool(name="est", bufs=4) as est_pool, \
                 tc.tile_pool(name="a1t", bufs=2) as a1t_pool, \
                 tc.tile_pool(name="small", bufs=4) as small, \
                 tc.tile_pool(name="outp", bufs=4) as outp:
                for b in range(NSQB):
                    bsl = slice(b * SQB, (b + 1) * SQB)
                    a1t = a1t_pool.tile([P, SQB], F32R, tag="a1t")
                    for h in range(HPC):
                        kp = kpadA if h == 0 else kpadB
                        at_ps = av_ps.tile([P, SQB], F32, tag="av")
                        for g in range(NCH // 2):
                            ps = sc_ps.tile([P, 2 * SQB], F32, tag="sc")
                            for j in range(2):
                                c = 2 * g + j
                                nc.tensor.matmul(
                                    ps[:, j * SQB : (j + 1) * SQB],
                                    lhsT=kp[:, c * P : (c + 1) * P],
                                    rhs=qt[:, bsl],
                                    start=True, stop=True,
                                )
                            es = est_pool.tile([P, 2 * SQB], BF16, tag="est")
                            nc.scalar.activation(es[:], ps[:], EXP, scale=float(SCALE))
                            for j in range(2):
                                c = 2 * g + j
                                nc.tensor.matmul(
                                    at_ps[0 : DV + 1, :],
                                    lhsT=vaug[h][:, c, :],
                                    rhs=es[:, j * SQB : (j + 1) * SQB],
                                    start=(c == 0), stop=(c == NCH - 1),
                                )
                        # normalize: A1T rows = A^T * (1/rowsum) broadcast
                        rsr = small.tile([1, SQB], F32, tag="rsr")
                        nc.vector.reciprocal(rsr[:], at_ps[DV : DV + 1, :])
                        bc = small.tile([P, SQB], F32, tag="bc")
                        nc.gpsimd.partition_broadcast(bc[0:DV, :], rsr[:])
                        if h == 0:
                            nc.vector.tensor_tensor(
                                a1t[0:DV, :], at_ps[0:DV, :], bc[0:DV, :], MULT
                            )
                        else:
                            tb = small.tile([P, SQB], F32R, tag="tb")
                            nc.vector.tensor_tensor(
                                tb[0:DV, :], at_ps[0:DV, :], bc[0:DV, :], MULT
                            )
                            nc.sync.dma_start(a1t[DV:P, :], tb[0:DV, :])

                    # output projection for this block
                    for j in range(NSQB):
                        rsl = slice(b * SQB + j * P, b * SQB + (j + 1) * P)
                        for e2 in range(E // SQB):
                            esl = slice(e2 * SQB, (e2 + 1) * SQB)
                            ops = av_ps.tile([P, SQB], F32, tag="av")
                            nc.tensor.matmul(
                                ops[:],
                                lhsT=a1t[:, j * P : (j + 1) * P],
                                rhs=wosb[:, esl],
                                start=True, stop=True,
                            )
                            osb = outp.tile([P, SQB], F32, tag="osb")
                            nc.vector.tensor_copy(osb[:], ops[:])
                            nc.sync.dma_start(y_ap[rsl, esl], osb[:])

    nc.compile()
    return nc


def kernel(x, Wq, Wk, Wv, Wo):
    global last_results
    x = np.ascontiguousarray(np.asarray(x, dtype=np.float32))
    Wq = np.asarray(Wq, dtype=np.float32)
    Wk = np.asarray(Wk, dtype=np.float32)
    Wv = np.asarray(Wv, dtype=np.float32)
    Wo = np.asarray(Wo, dtype=np.float32)

    if "nc" not in _cache:
        _cache["nc"] = _build_nc()
    nc = _cache["nc"]

    xT = np.ascontiguousarray(x.T)
    WqT = np.ascontiguousarray(Wq.T)
    WkT = np.ascontiguousarray(Wk.T)
    WvT = np.ascontiguousarray(Wv.T)
    WoT = np.ascontiguousarray(Wo.T)

    in_maps = []
    for i in range(NCORES):
        sl = slice(i * CSL, (i + 1) * CSL)
        in_maps.append({
            "xT": xT,
            "wqT": np.ascontiguousarray(WqT[:, sl]),
            "wkT": np.ascontiguousarray(WkT[:, sl]),
            "wvT": np.ascontiguousarray(WvT[:, sl]),
            "woT": np.ascontiguousarray(WoT[sl, :]),
        })

    last_results = run_bass_kernel_spmd(
        nc, in_maps, core_ids=list(range(NCORES)), trace=TRACE
    )
    out = np.zeros((S, E), dtype=np.float32)
    for r in last_results.results:
        out += r["y"]
    return out


# revision 29
# speedup vs baseline: 1.4007x; 1.4007x over previous
"""Multi-head attention TRN2 Bass kernel, head-sharded across 8 NeuronCores.

Problem: S=2048, E=1024, H=16 heads, dk=dv=64, fp32.
    Q = x @ Wq.T ; K = x @ Wk.T ; V = x @ Wv.T   (per-head slices)
    A_h = softmax(Q_h K_h^T / 8) V_h
    out = concat_h(A_h) @ Wo.T

Sharding: tensor-parallel over heads. Core i owns heads (2i, 2i+1), computes
its heads' Q/K/V projections over the full sequence, attention, and a partial
output projection against the matching 128-column slice of Wo. The 8 partial
[2048,1024] outputs are summed on the host (the gather/unshard step).

On-chip layout (everything "transposed" so no PE transposes of big tensors
are needed and softmax normalization rides the AV matmul as a ones column):
    xT   [E, S]   E on partitions (8 chunks of 128), bf16
    QT [128, S]: rows 0-63 head A, 64-127 head B  (dk on partitions)
    KT zero-padded per head so scores stay K=128 (single PE tile mode
    everywhere - mode switches drain the PE array)
    scores^T chunks [sk=128, sq] = kpad_h.T @ QT
    exp on ACT (scale=1/8 fused), bf16 out
    AV:  A^T[dv+1, sq] accumulated over chunks; lhsT = [V_chunk | ones],
         row 64 collects the softmax denominator for free
    outproj: lhsT = normalized A1^T [128, sq], rhs = WoT slice [128, E]

All matmul operands bf16 (empirically ~0.4% rel err vs fp64 reference,
fp32 PSUM accumulation throughout); fast 2-byte weight loads keep the PE
at ~1 cycle/row.
"""

import numpy as np
import ml_dtypes

import concourse.mybir as mybir
import concourse.tile as tile
from concourse import bacc
from concourse.bass_utils import run_bass_kernel_spmd

S, E, H, DK, DV = 2048, 1024, 16, 64, 64
NCORES = 8
HPC = H // NCORES          # heads per core = 2
CSL = HPC * DV             # concat-dim columns per core = 128
P = 128
NE = E // P                # 8 contraction chunks for projections
SQB = 512                  # sequence block (PSUM-bank-limited matmul width)
NSQB = S // SQB            # 4
NCH = S // P               # 16 sk chunks of 128
F32 = mybir.dt.float32
BF16 = mybir.dt.bfloat16
SCALE = 1.0 / np.sqrt(DK).astype(np.float32)  # 1/8

EXP = mybir.ActivationFunctionType.Exp
MULT = mybir.AluOpType.mult

_cache = {}
last_results = None  # BassKernelResults of the most recent run (for test.py)
TRACE = False


def _build_nc():
    nc = bacc.Bacc("TRN2", target_bir_lowering=False, debug=False)

    # host pre-arranges everything partition-major (and bf16) for fast DMA
    xT = nc.dram_tensor("xT", [P, NE, S], BF16, kind="ExternalInput")
    wqT = nc.dram_tensor("wqT", [P, NE, CSL], BF16, kind="ExternalInput")
    wkT = nc.dram_tensor("wkT", [P, NE, CSL], BF16, kind="ExternalInput")
    wvT = nc.dram_tensor("wvT", [P, NE, CSL], BF16, kind="ExternalInput")
    woT = nc.dram_tensor("woT", [CSL, E], BF16, kind="ExternalInput")
    y = nc.dram_tensor("y", [S, E], F32, kind="ExternalOutput")

    xT_r = xT.ap()
    w_r = {"q": wqT.ap(), "k": wkT.ap(), "v": wvT.ap()}
    y_ap = y.ap()

    with tile.TileContext(nc) as tc:
        with tc.tile_pool(name="persist", bufs=1) as persist, \
             tc.tile_pool(name="xw", bufs=1) as xw:
            # Persistent SBUF tensors
            qt = persist.tile([P, S], BF16)          # QT, both heads stacked
            kpad = [
                persist.tile([P, S], BF16, name=f"kpad{h}", tag=f"kpad{h}")
                for h in range(HPC)
            ]
            vaug = [
                persist.tile([P, NCH, DV + 2], BF16, name=f"vaug{h}", tag=f"vaug{h}")
                for h in range(HPC)
            ]
            wosb = persist.tile([P, E], BF16)

            # zero the unused half of each per-head padded KT, set ones cols
            nc.gpsimd.memset(kpad[0][DK:P, :], 0.0)
            nc.gpsimd.memset(kpad[1][0:DK, :], 0.0)
            for h in range(HPC):
                nc.gpsimd.memset(vaug[h][:, :, DV : DV + 2], 1.0)

            nc.sync.dma_start(wosb[:], woT.ap())
            wsb = {}
            for m in ("k", "q", "v"):
                wsb[m] = xw.tile([P, NE, CSL], BF16, name=f"w{m}sb", tag=f"w{m}")
                nc.sync.dma_start(wsb[m][:], w_r[m][:])
            xsb = xw.tile([P, NE, S], BF16)
            qs = [nc.scalar, nc.gpsimd, nc.sync]
            for n in range(NE):
                qs[n % 3].dma_start(xsb[:, n, :], xT_r[:, n, :])

            # ---- Phase B: K/Q projections (KT/QT = W_slice^T.T @ x^T) ----
            with tc.tile_pool(name="proj_ps", bufs=2, space="PSUM") as proj_ps:
                for m in ("k", "q"):
                    for t in range(NSQB):
                        sl = slice(t * SQB, (t + 1) * SQB)
                        ps = proj_ps.tile([P, SQB], F32, tag="proj")
                        for n in range(NE):
                            nc.tensor.matmul(
                                ps[:], lhsT=wsb[m][:, n, :], rhs=xsb[:, n, sl],
                                start=(n == 0), stop=(n == NE - 1),
                            )
                        if m == "q":
                            nc.vector.tensor_copy(qt[:, sl], ps[:])
                        else:
                            nc.vector.tensor_copy(kpad[0][0:DK, sl], ps[0:DK, :])
                            nc.vector.tensor_copy(kpad[1][DK:P, sl], ps[DK:P, :])

            # ---- Phase D: attention + output projection, per sq block ----
            # V is computed on the fly during block 0 (emit_v_chunk), directly
            # in [sk, dv] orientation: V chunk c = x[128c:128c+128] @ Wv^T.
            with tc.tile_pool(name="sc_ps", bufs=2, space="PSUM") as sc_ps, \
                 tc.tile_pool(name="av_ps", bufs=2, space="PSUM") as av_ps, \
                 tc.tile_pool(name="op_ps", bufs=2, space="PSUM") as op_ps, \
                 tc.tile_pool(name="est", bufs=8) as est_pool, \
                 tc.tile_pool(name="a1t", bufs=2) as a1t_pool, \
                 tc.tile_pool(name="small", bufs=4) as small, \
                 tc.tile_pool(name="outp", bufs=4) as outp:

                def emit_v_chunk(c):
                    vp = op_ps.tile([P, P], F32, name="vp", tag="op")
                    for n in range(NE):
                        nc.tensor.matmul(
                            vp[:],
                            lhsT=xsb[:, n, c * P : (c + 1) * P],
                            rhs=wsb["v"][:, n, :],
                            start=(n == 0), stop=(n == NE - 1),
                        )
                    nc.vector.tensor_copy(vaug[0][:, c, 0:DV], vp[:, 0:DV])
                    nc.vector.tensor_copy(vaug[1][:, c, 0:DV], vp[:, DV:P])

                for b in range(NSQB):
                    bsl = slice(b * SQB, (b + 1) * SQB)
                    a1t = a1t_pool.tile([P, SQB], BF16, tag="a1t")
                    at_ps = [
                        av_ps.tile([P, SQB], F32, name=f"at_ps{h}", tag="av")
                        for h in range(HPC)
                    ]
                    for g in range(NCH // 2):
                        if b == 0:
                            emit_v_chunk(2 * g)
                            emit_v_chunk(2 * g + 1)
                        pss = [
                            sc_ps.tile([P, 2 * SQB], F32, name=f"scps{h}", tag="sc")
                            for h in range(HPC)
                        ]
                        for j in range(2):
                            c = 2 * g + j
                            for h in range(HPC):
                                nc.tensor.matmul(
                                    pss[h][:, j * SQB : (j + 1) * SQB],
                                    lhsT=kpad[h][:, c * P : (c + 1) * P],
                                    rhs=qt[:, bsl],
                                    start=True, stop=True,
                                )
                        ess = []
                        for h in range(HPC):
                            es = est_pool.tile(
                                [P, 2 * SQB], BF16, name=f"est{h}", tag="est"
                            )
                            nc.scalar.activation(
                                es[:], pss[h][:], EXP, scale=float(SCALE)
                            )
                            ess.append(es)
                        for j in range(2):
                            c = 2 * g + j
                            for h in range(HPC):
                                nc.tensor.matmul(
                                    at_ps[h][0 : DV + 2, :],
                                    lhsT=vaug[h][:, c, :],
                                    rhs=ess[h][:, j * SQB : (j + 1) * SQB],
                                    start=(c == 0), stop=(c == NCH - 1),
                                )
                    # normalize: A1T rows = A^T * (1/rowsum) broadcast
                    for h in range(HPC):
                        rs0 = small.tile([1, SQB], F32, tag="rs0")
                        nc.vector.tensor_copy(rs0[:], at_ps[h][DV : DV + 1, :])
                        rsr = small.tile([1, SQB], F32, tag="rsr")
                        nc.vector.reciprocal_approx_fast(rsr[:], rs0[:])
                        bc = small.tile([P, SQB], F32, tag="bc")
                        nc.gpsimd.partition_broadcast(bc[0:DV, :], rsr[:])
                        if h == 0:
                            nc.vector.tensor_tensor(
                                a1t[0:DV, :], at_ps[h][0:DV, :], bc[0:DV, :], MULT
                            )
                        else:
                            tb = small.tile([P, SQB], BF16, tag="tb")
                            nc.vector.tensor_tensor(
                                tb[0:DV, :], at_ps[h][0:DV, :], bc[0:DV, :], MULT
                            )
                            nc.gpsimd.dma_start(a1t[DV:P, :], tb[0:DV, :])

                    # output projection for this block
                    for j in range(NSQB):
                        rsl = slice(b * SQB + j * P, b * SQB + (j + 1) * P)
                        osb = outp.tile([P, E], F32, tag="osb")
                        for e2 in range(E // SQB):
                            esl = slice(e2 * SQB, (e2 + 1) * SQB)
                            if b == NSQB - 1:
                                ops = sc_ps.tile(
                                    [P, SQB], F32, name="ops2", tag="sc"
                                )
                            else:
                                ops = op_ps.tile(
                                    [P, SQB], F32, name="ops", tag="op"
                                )
                            nc.tensor.matmul(
                                ops[:],
                                lhsT=a1t[:, j * P : (j + 1) * P],
                                rhs=wosb[:, esl],
                                start=True, stop=True,
                            )
                            nc.vector.tensor_copy(osb[:, esl], ops[:])
                        nc.sync.dma_start(y_ap[rsl, :], osb[:])

    nc.compile()
    return nc


def kernel(x, Wq, Wk, Wv, Wo):
    global last_results
    x = np.asarray(x, dtype=np.float32)
    Wq = np.asarray(Wq, dtype=np.float32)
    Wk = np.asarray(Wk, dtype=np.float32)
    Wv = np.asarray(Wv, dtype=np.float32)
    Wo = np.asarray(Wo, dtype=np.float32)

    if "nc" not in _cache:
        _cache["nc"] = _build_nc()
    nc = _cache["nc"]

    bf = ml_dtypes.bfloat16
    # [E, S] -> [P, NE, S] partition-major (chunk n, partition p = row n*P+p)
    xT = np.ascontiguousarray(
        x.T.reshape(NE, P, S).transpose(1, 0, 2).astype(bf)
    )
    WqT = np.ascontiguousarray(Wq.T)
    WkT = np.ascontiguousarray(Wk.T)
    WvT = np.ascontiguousarray(Wv.T)
    WoT = np.ascontiguousarray(Wo.T)

    in_maps = []
    for i in range(NCORES):
        sl = slice(i * CSL, (i + 1) * CSL)

        def wslice(WT):
            # [E, CSL] slice -> [P, NE, CSL] partition-major
            return np.ascontiguousarray(
                WT[:, sl].reshape(NE, P, CSL).transpose(1, 0, 2).astype(bf)
            )

        in_maps.append({
            "xT": xT,
            "wqT": wslice(WqT),
            "wkT": wslice(WkT),
            "wvT": wslice(WvT),
            "woT": np.ascontiguousarray(WoT[sl, :].astype(bf)),
        })

    last_results = run_bass_kernel_spmd(
        nc, in_maps, core_ids=list(range(NCORES)), trace=TRACE
    )
    out = np.zeros((S, E), dtype=np.float32)
    for r in last_results.results:
        out += r["y"]
    return out


# revision 30
# speedup vs baseline: 1.4367x; 1.0257x over previous
"""Multi-head attention TRN2 Bass kernel, head-sharded across 8 NeuronCores.

Problem: S=2048, E=1024, H=16 heads, dk=dv=64, fp32.
    Q = x @ Wq.T ; K = x @ Wk.T ; V = x @ Wv.T   (per-head slices)
    A_h = softmax(Q_h K_h^T / 8) V_h
    out = concat_h(A_h) @ Wo.T

Sharding: tensor-parallel over heads. Core i owns heads (2i, 2i+1), computes
its heads' Q/K/V projections over the full sequence, attention, and a partial
output projection against the matching 128-column slice of Wo. The 8 partial
[2048,1024] outputs are summed on the host (the gather/unshard step).

On-chip layout (everything "transposed" so no PE transposes of big tensors
are needed and softmax normalization rides the AV matmul as a ones column):
    xT   [E, S]   E on partitions (8 chunks of 128), bf16
    QT [128, S]: rows 0-63 head A, 64-127 head B  (dk on partitions)
    KT zero-padded per head so scores stay K=128 (single PE tile mode
    everywhere - mode switches drain the PE array)
    scores^T chunks [sk=128, sq] = kpad_h.T @ QT
    exp on ACT (scale=1/8 fused), bf16 out
    AV:  A^T[dv+1, sq] accumulated over chunks; lhsT = [V_chunk | ones],
         row 64 collects the softmax denominator for free
    outproj: lhsT = normalized A1^T [128, sq], rhs = WoT slice [128, E]

All matmul operands bf16 (empirically ~0.4% rel err vs fp64 reference,
fp32 PSUM accumulation throughout); fast 2-byte weight loads keep the PE
at ~1 cycle/row.
"""

import numpy as np
import ml_dtypes

import concourse.mybir as mybir
import concourse.tile as tile
from concourse import bacc
from concourse.bass_utils import run_bass_kernel_spmd

S, E, H, DK, DV = 2048, 1024, 16, 64, 64
NCORES = 8
HPC = H // NCORES          # heads per core = 2
CSL = HPC * DV             # concat-dim columns per core = 128
P = 128
NE = E // P                # 8 contraction chunks for projections
SQB = 512                  # sequence block (PSUM-bank-limited matmul width)
NSQB = S // SQB            # 4
NCH = S // P               # 16 sk chunks of 128
F32 = mybir.dt.float32
BF16 = mybir.dt.bfloat16
SCALE = 1.0 / np.sqrt(DK).astype(np.float32)  # 1/8

EXP = mybir.ActivationFunctionType.Exp
MULT = mybir.AluOpType.mult

_cache = {}
last_results = None  # BassKernelResults of the most recent run (for test.py)
TRACE = False


def _build_nc():
    nc = bacc.Bacc("TRN2", target_bir_lowering=False, debug=False)

    # host pre-arranges everything partition-major (and bf16) for fast DMA
    xT = nc.dram_tensor("xT", [P, NE, S], BF16, kind="ExternalInput")
    wqT = nc.dram_tensor("wqT", [P, NE, CSL], BF16, kind="ExternalInput")
    wkT = nc.dram_tensor("wkT", [P, NE, CSL], BF16, kind="ExternalInput")
    wvT = nc.dram_tensor("wvT", [P, NE, CSL], BF16, kind="ExternalInput")
    woT = nc.dram_tensor("woT", [CSL, E], BF16, kind="ExternalInput")
    y = nc.dram_tensor("y", [S, E], F32, kind="ExternalOutput")

    xT_r = xT.ap()
    w_r = {"q": wqT.ap(), "k": wkT.ap(), "v": wvT.ap()}
    y_ap = y.ap()

    with tile.TileContext(nc) as tc:
        with tc.tile_pool(name="persist", bufs=1) as persist, \
             tc.tile_pool(name="xw", bufs=1) as xw:
            # Persistent SBUF tensors
            qt = persist.tile([P, S], BF16)          # QT, both heads stacked
            kpad = [
                persist.tile([P, S], BF16, name=f"kpad{h}", tag=f"kpad{h}")
                for h in range(HPC)
            ]
            vaug = [
                persist.tile([P, NCH, DV + 2], BF16, name=f"vaug{h}", tag=f"vaug{h}")
                for h in range(HPC)
            ]
            wosb = persist.tile([P, E], BF16)

            # zero the unused half of each per-head padded KT, set ones cols
            nc.gpsimd.memset(kpad[0][DK:P, :], 0.0)
            nc.gpsimd.memset(kpad[1][0:DK, :], 0.0)
            for h in range(HPC):
                nc.gpsimd.memset(vaug[h][:, :, DV : DV + 2], 1.0)

            nc.sync.dma_start(wosb[:], woT.ap())
            wsb = {}
            for m in ("k", "q", "v"):
                wsb[m] = xw.tile([P, NE, CSL], BF16, name=f"w{m}sb", tag=f"w{m}")
                nc.sync.dma_start(wsb[m][:], w_r[m][:])
            xsb = xw.tile([P, NE, S], BF16)
            qs = [nc.scalar, nc.gpsimd, nc.sync]
            for n in range(NE):
                qs[n % 3].dma_start(xsb[:, n, :], xT_r[:, n, :])

            # ---- Phase B: K/Q projections (KT/QT = W_slice^T.T @ x^T) ----
            with tc.tile_pool(name="proj_ps", bufs=2, space="PSUM") as proj_ps:
                for m in ("k", "q"):
                    for t in range(NSQB):
                        sl = slice(t * SQB, (t + 1) * SQB)
                        ps = proj_ps.tile([P, SQB], F32, tag="proj")
                        for n in range(NE):
                            nc.tensor.matmul(
                                ps[:], lhsT=wsb[m][:, n, :], rhs=xsb[:, n, sl],
                                start=(n == 0), stop=(n == NE - 1),
                            )
                        if m == "q":
                            nc.vector.tensor_copy(qt[:, sl], ps[:])
                        else:
                            nc.vector.tensor_copy(kpad[0][0:DK, sl], ps[0:DK, :])
                            nc.vector.tensor_copy(kpad[1][DK:P, sl], ps[DK:P, :])

            # ---- Phase D: attention + output projection, per sq block ----
            # V is computed on the fly during block 0 (emit_v_chunk), directly
            # in [sk, dv] orientation: V chunk c = x[128c:128c+128] @ Wv^T.
            with tc.tile_pool(name="sc_ps", bufs=2, space="PSUM") as sc_ps, \
                 tc.tile_pool(name="av_ps", bufs=3, space="PSUM") as av_ps, \
                 tc.tile_pool(name="op_ps", bufs=1, space="PSUM") as op_ps, \
                 tc.tile_pool(name="est", bufs=8) as est_pool, \
                 tc.tile_pool(name="a1t", bufs=2) as a1t_pool, \
                 tc.tile_pool(name="small", bufs=4) as small, \
                 tc.tile_pool(name="outp", bufs=4) as outp:

                def emit_v_chunk(c):
                    vp = op_ps.tile([P, P], F32, name="vp", tag="op")
                    for n in range(NE):
                        nc.tensor.matmul(
                            vp[:],
                            lhsT=xsb[:, n, c * P : (c + 1) * P],
                            rhs=wsb["v"][:, n, :],
                            start=(n == 0), stop=(n == NE - 1),
                        )
                    nc.vector.tensor_copy(vaug[0][:, c, 0:DV], vp[:, 0:DV])
                    nc.vector.tensor_copy(vaug[1][:, c, 0:DV], vp[:, DV:P])

                for b in range(NSQB):
                    bsl = slice(b * SQB, (b + 1) * SQB)
                    a1t = a1t_pool.tile([P, SQB], BF16, tag="a1t")
                    at_ps = [
                        av_ps.tile([P, SQB], F32, name=f"at_ps{h}", tag="av")
                        for h in range(HPC)
                    ]
                    for g in range(NCH // 2):
                        if b == 0:
                            emit_v_chunk(2 * g)
                            emit_v_chunk(2 * g + 1)
                        pss = [
                            sc_ps.tile([P, 2 * SQB], F32, name=f"scps{h}", tag="sc")
                            for h in range(HPC)
                        ]
                        for j in range(2):
                            c = 2 * g + j
                            for h in range(HPC):
                                nc.tensor.matmul(
                                    pss[h][:, j * SQB : (j + 1) * SQB],
                                    lhsT=kpad[h][:, c * P : (c + 1) * P],
                                    rhs=qt[:, bsl],
                                    start=True, stop=True,
                                )
                        ess = []
                        for h in range(HPC):
                            es = est_pool.tile(
                                [P, 2 * SQB], BF16, name=f"est{h}", tag="est"
                            )
                            nc.scalar.activation(
                                es[:], pss[h][:], EXP, scale=float(SCALE)
                            )
                            ess.append(es)
                        for j in range(2):
                            c = 2 * g + j
                            for h in range(HPC):
                                nc.tensor.matmul(
                                    at_ps[h][0 : DV + 2, :],
                                    lhsT=vaug[h][:, c, :],
                                    rhs=ess[h][:, j * SQB : (j + 1) * SQB],
                                    start=(c == 0), stop=(c == NCH - 1),
                                )
                    # normalize: A1T rows = A^T * (1/rowsum) broadcast
                    for h in range(HPC):
                        rs0 = small.tile([1, SQB], F32, tag="rs0")
                        nc.vector.tensor_copy(rs0[:], at_ps[h][DV : DV + 1, :])
                        rsr = small.tile([1, SQB], F32, tag="rsr")
                        nc.vector.reciprocal_approx_fast(rsr[:], rs0[:])
                        bc = small.tile([P, SQB], F32, tag="bc")
                        nc.gpsimd.partition_broadcast(bc[0:DV, :], rsr[:])
                        if h == 0:
                            nc.vector.tensor_tensor(
                                a1t[0:DV, :], at_ps[h][0:DV, :], bc[0:DV, :], MULT
                            )
                        else:
                            tb = small.tile([P, SQB], BF16, tag="tb")
                            nc.vector.tensor_tensor(
                                tb[0:DV, :], at_ps[h][0:DV, :], bc[0:DV, :], MULT
                            )
                            nc.gpsimd.dma_start(a1t[DV:P, :], tb[0:DV, :])

                    # output projection for this block
                    for j in range(NSQB):
                        rsl = slice(b * SQB + j * P, b * SQB + (j + 1) * P)
                        osb = outp.tile([P, E], F32, tag="osb")
                        for e2 in range(E // SQB):
                            esl = slice(e2 * SQB, (e2 + 1) * SQB)
                            if b == NSQB - 1:
                                ops = sc_ps.tile(
                                    [P, SQB], F32, name="ops2", tag="sc"
                                )
                            else:
                                ops = op_ps.tile(
                                    [P, SQB], F32, name="ops", tag="op"
                                )
                            nc.tensor.matmul(
                                ops[:],
                                lhsT=a1t[:, j * P : (j + 1) * P],
                                rhs=wosb[:, esl],
                                start=True, stop=True,
                            )
                            nc.vector.tensor_copy(osb[:, esl], ops[:])
                        nc.sync.dma_start(y_ap[rsl, :], osb[:])

    nc.compile()
    return nc


def kernel(x, Wq, Wk, Wv, Wo):
    global last_results
    x = np.asarray(x, dtype=np.float32)
    Wq = np.asarray(Wq, dtype=np.float32)
    Wk = np.asarray(Wk, dtype=np.float32)
    Wv = np.asarray(Wv, dtype=np.float32)
    Wo = np.asarray(Wo, dtype=np.float32)

    if "nc" not in _cache:
        _cache["nc"] = _build_nc()
    nc = _cache["nc"]

    bf = ml_dtypes.bfloat16
    # [E, S] -> [P, NE, S] partition-major (chunk n, partition p = row n*P+p)
    xT = np.ascontiguousarray(
        x.T.reshape(NE, P, S).transpose(1, 0, 2).astype(bf)
    )
    WqT = np.ascontiguousarray(Wq.T)
    WkT = np.ascontiguousarray(Wk.T)
    WvT = np.ascontiguousarray(Wv.T)
    WoT = np.ascontiguousarray(Wo.T)

    in_maps = []
    for i in range(NCORES):
        sl = slice(i * CSL, (i + 1) * CSL)

        def wslice(WT):
            # [E, CSL] slice -> [P, NE, CSL] partition-major
            return np.ascontiguousarray(
                WT[:, sl].reshape(NE, P, CSL).transpose(1, 0, 2).astype(bf)
            )

        in_maps.append({
            "xT": xT,
            "wqT": wslice(WqT),
            "wkT": wslice(WkT),
            "wvT": wslice(WvT),
            "woT": np.ascontiguousarray(WoT[sl, :].astype(bf)),
        })

    last_results = run_bass_kernel_spmd(
        nc, in_maps, core_ids=list(range(NCORES)), trace=TRACE
    )
    out = np.zeros((S, E), dtype=np.float32)
    for r in last_results.results:
        out += r["y"]
    return out


# revision 31
# speedup vs baseline: 1.4392x; 1.0017x over previous
"""Multi-head attention TRN2 Bass kernel, head-sharded across 8 NeuronCores.

Problem: S=2048, E=1024, H=16 heads, dk=dv=64, fp32.
    Q = x @ Wq.T ; K = x @ Wk.T ; V = x @ Wv.T   (per-head slices)
    A_h = softmax(Q_h K_h^T / 8) V_h
    out = concat_h(A_h) @ Wo.T

Sharding: tensor-parallel over heads. Core i owns heads (2i, 2i+1), computes
its heads' Q/K/V projections over the full sequence, attention, and a partial
output projection against the matching 128-column slice of Wo. The 8 partial
[2048,1024] outputs are summed on the host (the gather/unshard step).

On-chip layout (everything "transposed" so no PE transposes of big tensors
are needed and softmax normalization rides the AV matmul as a ones column):
    xT   [E, S]   E on partitions (8 chunks of 128), bf16
    QT [128, S]: rows 0-63 head A, 64-127 head B  (dk on partitions)
    KT zero-padded per head so scores stay K=128 (single PE tile mode
    everywhere - mode switches drain the PE array)
    scores^T chunks [sk=128, sq] = kpad_h.T @ QT
    exp on ACT (scale=1/8 fused), bf16 out
    AV:  A^T[dv+1, sq] accumulated over chunks; lhsT = [V_chunk | ones],
         row 64 collects the softmax denominator for free
    outproj: lhsT = normalized A1^T [128, sq], rhs = WoT slice [128, E]

All matmul operands bf16 (empirically ~0.4% rel err vs fp64 reference,
fp32 PSUM accumulation throughout); fast 2-byte weight loads keep the PE
at ~1 cycle/row.
"""

import numpy as np
import ml_dtypes

import concourse.mybir as mybir
import concourse.tile as tile
from concourse import bacc
from concourse.bass_utils import run_bass_kernel_spmd

S, E, H, DK, DV = 2048, 1024, 16, 64, 64
NCORES = 8
HPC = H // NCORES          # heads per core = 2
CSL = HPC * DV             # concat-dim columns per core = 128
P = 128
NE = E // P                # 8 contraction chunks for projections
SQB = 512                  # sequence block (PSUM-bank-limited matmul width)
NSQB = S // SQB            # 4
NCH = S // P               # 16 sk chunks of 128
F32 = mybir.dt.float32
BF16 = mybir.dt.bfloat16
SCALE = 1.0 / np.sqrt(DK).astype(np.float32)  # 1/8

EXP = mybir.ActivationFunctionType.Exp
MULT = mybir.AluOpType.mult

_cache = {}
last_results = None  # BassKernelResults of the most recent run (for test.py)
TRACE = False


def _build_nc():
    nc = bacc.Bacc("TRN2", target_bir_lowering=False, debug=False)

    # host pre-arranges everything partition-major (and bf16) for fast DMA
    xT = nc.dram_tensor("xT", [P, NE, S], BF16, kind="ExternalInput")
    wqT = nc.dram_tensor("wqT", [P, NE, CSL], BF16, kind="ExternalInput")
    wkT = nc.dram_tensor("wkT", [P, NE, CSL], BF16, kind="ExternalInput")
    wvT = nc.dram_tensor("wvT", [P, NE, CSL], BF16, kind="ExternalInput")
    woT = nc.dram_tensor("woT", [CSL, E], BF16, kind="ExternalInput")
    y = nc.dram_tensor("y", [S, E], F32, kind="ExternalOutput")

    xT_r = xT.ap()
    w_r = {"q": wqT.ap(), "k": wkT.ap(), "v": wvT.ap()}
    y_ap = y.ap()

    with tile.TileContext(nc) as tc:
        with tc.tile_pool(name="persist", bufs=1) as persist, \
             tc.tile_pool(name="xw", bufs=1) as xw:
            # Persistent SBUF tensors
            qt = persist.tile([P, S], BF16)          # QT, both heads stacked
            kpad = [
                persist.tile([P, S], BF16, name=f"kpad{h}", tag=f"kpad{h}")
                for h in range(HPC)
            ]
            vaug = [
                persist.tile([P, NCH, DV + 2], BF16, name=f"vaug{h}", tag=f"vaug{h}")
                for h in range(HPC)
            ]
            wosb = persist.tile([P, E], BF16)

            # zero the unused half of each per-head padded KT, set ones cols
            nc.gpsimd.memset(kpad[0][DK:P, :], 0.0)
            nc.gpsimd.memset(kpad[1][0:DK, :], 0.0)
            for h in range(HPC):
                nc.gpsimd.memset(vaug[h][:, :, DV : DV + 2], 1.0)

            nc.sync.dma_start(wosb[:], woT.ap())
            wsb = {}
            for m in ("k", "q", "v"):
                wsb[m] = xw.tile([P, NE, CSL], BF16, name=f"w{m}sb", tag=f"w{m}")
                nc.sync.dma_start(wsb[m][:], w_r[m][:])
            xsb = xw.tile([P, NE, S], BF16)
            qs = [nc.scalar, nc.gpsimd, nc.sync]
            for n in range(NE):
                qs[n % 3].dma_start(xsb[:, n, :], xT_r[:, n, :])

            # ---- Phase B: K/Q projections (KT/QT = W_slice^T.T @ x^T) ----
            with tc.tile_pool(name="proj_ps", bufs=2, space="PSUM") as proj_ps:
                for m in ("k", "q"):
                    for t in range(NSQB):
                        sl = slice(t * SQB, (t + 1) * SQB)
                        ps = proj_ps.tile([P, SQB], F32, tag="proj")
                        for n in range(NE):
                            nc.tensor.matmul(
                                ps[:], lhsT=wsb[m][:, n, :], rhs=xsb[:, n, sl],
                                start=(n == 0), stop=(n == NE - 1),
                            )
                        if m == "q":
                            nc.vector.tensor_copy(qt[:, sl], ps[:])
                        else:
                            nc.vector.tensor_copy(kpad[0][0:DK, sl], ps[0:DK, :])
                            nc.vector.tensor_copy(kpad[1][DK:P, sl], ps[DK:P, :])

            # ---- Phase D: attention + output projection, per sq block ----
            # V is computed on the fly during block 0 (emit_v_chunk), directly
            # in [sk, dv] orientation: V chunk c = x[128c:128c+128] @ Wv^T.
            with tc.tile_pool(name="sc_ps", bufs=2, space="PSUM") as sc_ps, \
                 tc.tile_pool(name="av_ps", bufs=3, space="PSUM") as av_ps, \
                 tc.tile_pool(name="op_ps", bufs=1, space="PSUM") as op_ps, \
                 tc.tile_pool(name="est", bufs=12) as est_pool, \
                 tc.tile_pool(name="a1t", bufs=2) as a1t_pool, \
                 tc.tile_pool(name="small", bufs=6) as small, \
                 tc.tile_pool(name="outp", bufs=6) as outp:

                def emit_v_chunk(c):
                    vp = op_ps.tile([P, P], F32, name="vp", tag="op")
                    for n in range(NE):
                        nc.tensor.matmul(
                            vp[:],
                            lhsT=xsb[:, n, c * P : (c + 1) * P],
                            rhs=wsb["v"][:, n, :],
                            start=(n == 0), stop=(n == NE - 1),
                        )
                    nc.vector.tensor_copy(vaug[0][:, c, 0:DV], vp[:, 0:DV])
                    nc.vector.tensor_copy(vaug[1][:, c, 0:DV], vp[:, DV:P])

                for b in range(NSQB):
                    bsl = slice(b * SQB, (b + 1) * SQB)
                    a1t = a1t_pool.tile([P, SQB], BF16, tag="a1t")
                    at_ps = [
                        av_ps.tile([P, SQB], F32, name=f"at_ps{h}", tag="av")
                        for h in range(HPC)
                    ]
                    for g in range(NCH // 2):
                        if b == 0:
                            emit_v_chunk(2 * g)
                            emit_v_chunk(2 * g + 1)
                        pss = [
                            sc_ps.tile([P, 2 * SQB], F32, name=f"scps{h}", tag="sc")
                            for h in range(HPC)
                        ]
                        for j in range(2):
                            c = 2 * g + j
                            for h in range(HPC):
                                nc.tensor.matmul(
                                    pss[h][:, j * SQB : (j + 1) * SQB],
                                    lhsT=kpad[h][:, c * P : (c + 1) * P],
                                    rhs=qt[:, bsl],
                                    start=True, stop=True,
                                )
                        ess = []
                        for h in range(HPC):
                            es = est_pool.tile(
                                [P, 2 * SQB], BF16, name=f"est{h}", tag="est"
                            )
                            nc.scalar.activation(
                                es[:], pss[h][:], EXP, scale=float(SCALE)
                            )
                            ess.append(es)
                        for j in range(2):
                            c = 2 * g + j
                            for h in range(HPC):
                                nc.tensor.matmul(
                                    at_ps[h][0 : DV + 2, :],
                                    lhsT=vaug[h][:, c, :],
                                    rhs=ess[h][:, j * SQB : (j + 1) * SQB],
                                    start=(c == 0), stop=(c == NCH - 1),
                                )
                    # normalize: A1T rows = A^T * (1/rowsum) broadcast
                    for h in range(HPC):
                        rs0 = small.tile([1, SQB], F32, tag="rs0")
                        nc.vector.tensor_copy(rs0[:], at_ps[h][DV : DV + 1, :])
                        rsr = small.tile([1, SQB], F32, tag="rsr")
                        nc.vector.reciprocal_approx_fast(rsr[:], rs0[:])
                        bc = small.tile([P, SQB], F32, tag="bc")
                        nc.gpsimd.partition_broadcast(bc[0:DV, :], rsr[:])
                        if h == 0:
                            nc.vector.tensor_tensor(
                                a1t[0:DV, :], at_ps[h][0:DV, :], bc[0:DV, :], MULT
                            )
                        else:
                            tb = small.tile([P, SQB], BF16, tag="tb")
                            nc.vector.tensor_tensor(
                                tb[0:DV, :], at_ps[h][0:DV, :], bc[0:DV, :], MULT
                            )
                            nc.gpsimd.dma_start(a1t[DV:P, :], tb[0:DV, :])

                    # output projection for this block
                    for j in range(NSQB):
                        rsl = slice(b * SQB + j * P, b * SQB + (j + 1) * P)
                        osb = outp.tile([P, E], F32, tag="osb")
                        for e2 in range(E // SQB):
                            esl = slice(e2 * SQB, (e2 + 1) * SQB)
                            if b == NSQB - 1:
                                ops = sc_ps.tile(
                                    [P, SQB], F32, name="ops2", tag="sc"
                                )
                            else:
                                ops = op_ps.tile(
                                    [P, SQB], F32, name="ops", tag="op"
                                )
                            nc.tensor.matmul(
                                ops[:],
                                lhsT=a1t[:, j * P : (j + 1) * P],
                                rhs=wosb[:, esl],
                                start=True, stop=True,
                            )
                            nc.vector.tensor_copy(osb[:, esl], ops[:])
                        nc.sync.dma_start(y_ap[rsl, :], osb[:])

    nc.compile()
    return nc


def kernel(x, Wq, Wk, Wv, Wo):
    global last_results
    x = np.asarray(x, dtype=np.float32)
    Wq = np.asarray(Wq, dtype=np.float32)
    Wk = np.asarray(Wk, dtype=np.float32)
    Wv = np.asarray(Wv, dtype=np.float32)
    Wo = np.asarray(Wo, dtype=np.float32)

    if "nc" not in _cache:
        _cache["nc"] = _build_nc()
    nc = _cache["nc"]

    bf = ml_dtypes.bfloat16
    # [E, S] -> [P, NE, S] partition-major (chunk n, partition p = row n*P+p)
    xT = np.ascontiguousarray(
        x.T.reshape(NE, P, S).transpose(1, 0, 2).astype(bf)
    )
    WqT = np.ascontiguousarray(Wq.T)
    WkT = np.ascontiguousarray(Wk.T)
    WvT = np.ascontiguousarray(Wv.T)
    WoT = np.ascontiguousarray(Wo.T)

    in_maps = []
    for i in range(NCORES):
        sl = slice(i * CSL, (i + 1) * CSL)

        def wslice(WT):
            # [E, CSL] slice -> [P, NE, CSL] partition-major
            return np.ascontiguousarray(
                WT[:, sl].reshape(NE, P, CSL).transpose(1, 0, 2).astype(bf)
            )

        in_maps.append({
            "xT": xT,
            "wqT": wslice(WqT),
            "wkT": wslice(WkT),
            "wvT": wslice(WvT),
            "woT": np.ascontiguousarray(WoT[sl, :].astype(bf)),
        })

    last_results = run_bass_kernel_spmd(
        nc, in_maps, core_ids=list(range(NCORES)), trace=TRACE
    )
    out = np.zeros((S, E), dtype=np.float32)
    for r in last_results.results:
        out += r["y"]
    return out


# revision 32
# speedup vs baseline: 1.4565x; 1.0120x over previous
"""Multi-head attention TRN2 Bass kernel, head-sharded across 8 NeuronCores.

Problem: S=2048, E=1024, H=16 heads, dk=dv=64, fp32.
    Q = x @ Wq.T ; K = x @ Wk.T ; V = x @ Wv.T   (per-head slices)
    A_h = softmax(Q_h K_h^T / 8) V_h
    out = concat_h(A_h) @ Wo.T

Sharding: tensor-parallel over heads. Core i owns heads (2i, 2i+1), computes
its heads' Q/K/V projections over the full sequence, attention, and a partial
output projection against the matching 128-column slice of Wo. The 8 partial
[2048,1024] outputs are summed on the host (the gather/unshard step).

On-chip layout (everything "transposed" so no PE transposes of big tensors
are needed and softmax normalization rides the AV matmul as a ones column):
    xT   [E, S]   E on partitions (8 chunks of 128), bf16
    QT [128, S]: rows 0-63 head A, 64-127 head B  (dk on partitions)
    KT zero-padded per head so scores stay K=128 (single PE tile mode
    everywhere - mode switches drain the PE array)
    scores^T chunks [sk=128, sq] = kpad_h.T @ QT
    exp on ACT (scale=1/8 fused), bf16 out
    AV:  A^T[dv+1, sq] accumulated over chunks; lhsT = [V_chunk | ones],
         row 64 collects the softmax denominator for free
    outproj: lhsT = normalized A1^T [128, sq], rhs = WoT slice [128, E]

All matmul operands bf16 (empirically ~0.4% rel err vs fp64 reference,
fp32 PSUM accumulation throughout); fast 2-byte weight loads keep the PE
at ~1 cycle/row.
"""

import numpy as np
import ml_dtypes

import concourse.mybir as mybir
import concourse.tile as tile
from concourse import bacc
from concourse.bass_utils import run_bass_kernel_spmd

S, E, H, DK, DV = 2048, 1024, 16, 64, 64
NCORES = 8
HPC = H // NCORES          # heads per core = 2
CSL = HPC * DV             # concat-dim columns per core = 128
P = 128
NE = E // P                # 8 contraction chunks for projections
SQB = 512                  # sequence block (PSUM-bank-limited matmul width)
NSQB = S // SQB            # 4
NCH = S // P               # 16 sk chunks of 128
F32 = mybir.dt.float32
BF16 = mybir.dt.bfloat16
SCALE = 1.0 / np.sqrt(DK).astype(np.float32)  # 1/8

EXP = mybir.ActivationFunctionType.Exp
MULT = mybir.AluOpType.mult

_cache = {}
last_results = None  # BassKernelResults of the most recent run (for test.py)
TRACE = False


def _build_nc():
    nc = bacc.Bacc("TRN2", target_bir_lowering=False, debug=False)

    # host pre-arranges everything partition-major (and bf16) for fast DMA
    xT = nc.dram_tensor("xT", [P, NE, S], BF16, kind="ExternalInput")
    wqT = nc.dram_tensor("wqT", [P, NE, CSL], BF16, kind="ExternalInput")
    wkT = nc.dram_tensor("wkT", [P, NE, CSL], BF16, kind="ExternalInput")
    wvT = nc.dram_tensor("wvT", [P, NE, CSL], BF16, kind="ExternalInput")
    woT = nc.dram_tensor("woT", [CSL, E], BF16, kind="ExternalInput")
    y = nc.dram_tensor("y", [S, E], BF16, kind="ExternalOutput")

    xT_r = xT.ap()
    w_r = {"q": wqT.ap(), "k": wkT.ap(), "v": wvT.ap()}
    y_ap = y.ap()

    with tile.TileContext(nc) as tc:
        with tc.tile_pool(name="persist", bufs=1) as persist, \
             tc.tile_pool(name="xw", bufs=1) as xw:
            # Persistent SBUF tensors
            qt = persist.tile([P, S], BF16)          # QT, both heads stacked
            kpad = [
                persist.tile([P, S], BF16, name=f"kpad{h}", tag=f"kpad{h}")
                for h in range(HPC)
            ]
            vaug = [
                persist.tile([P, NCH, DV + 2], BF16, name=f"vaug{h}", tag=f"vaug{h}")
                for h in range(HPC)
            ]
            wosb = persist.tile([P, E], BF16)

            # zero the unused half of each per-head padded KT, set ones cols
            nc.gpsimd.memset(kpad[0][DK:P, :], 0.0)
            nc.gpsimd.memset(kpad[1][0:DK, :], 0.0)
            for h in range(HPC):
                nc.gpsimd.memset(vaug[h][:, :, DV : DV + 2], 1.0)

            nc.sync.dma_start(wosb[:], woT.ap())
            wsb = {}
            for m in ("k", "q", "v"):
                wsb[m] = xw.tile([P, NE, CSL], BF16, name=f"w{m}sb", tag=f"w{m}")
                nc.sync.dma_start(wsb[m][:], w_r[m][:])
            xsb = xw.tile([P, NE, S], BF16)
            qs = [nc.scalar, nc.gpsimd, nc.sync]
            for n in range(NE):
                qs[n % 3].dma_start(xsb[:, n, :], xT_r[:, n, :])

            # ---- Phase B: K/Q projections (KT/QT = W_slice^T.T @ x^T) ----
            with tc.tile_pool(name="proj_ps", bufs=2, space="PSUM") as proj_ps:
                for m in ("k", "q"):
                    for t in range(NSQB):
                        sl = slice(t * SQB, (t + 1) * SQB)
                        ps = proj_ps.tile([P, SQB], F32, tag="proj")
                        for n in range(NE):
                            nc.tensor.matmul(
                                ps[:], lhsT=wsb[m][:, n, :], rhs=xsb[:, n, sl],
                                start=(n == 0), stop=(n == NE - 1),
                            )
                        if m == "q":
                            nc.vector.tensor_copy(qt[:, sl], ps[:])
                        else:
                            nc.vector.tensor_copy(kpad[0][0:DK, sl], ps[0:DK, :])
                            nc.vector.tensor_copy(kpad[1][DK:P, sl], ps[DK:P, :])

            # ---- Phase D: attention + output projection, per sq block ----
            # V is computed on the fly during block 0 (emit_v_chunk), directly
            # in [sk, dv] orientation: V chunk c = x[128c:128c+128] @ Wv^T.
            with tc.tile_pool(name="sc_ps", bufs=2, space="PSUM") as sc_ps, \
                 tc.tile_pool(name="av_ps", bufs=3, space="PSUM") as av_ps, \
                 tc.tile_pool(name="op_ps", bufs=1, space="PSUM") as op_ps, \
                 tc.tile_pool(name="est", bufs=12) as est_pool, \
                 tc.tile_pool(name="a1t", bufs=2) as a1t_pool, \
                 tc.tile_pool(name="small", bufs=6) as small, \
                 tc.tile_pool(name="outp", bufs=6) as outp:

                def emit_v_chunk(c):
                    vp = op_ps.tile([P, P], F32, name="vp", tag="op")
                    for n in range(NE):
                        nc.tensor.matmul(
                            vp[:],
                            lhsT=xsb[:, n, c * P : (c + 1) * P],
                            rhs=wsb["v"][:, n, :],
                            start=(n == 0), stop=(n == NE - 1),
                        )
                    nc.vector.tensor_copy(vaug[0][:, c, 0:DV], vp[:, 0:DV])
                    nc.vector.tensor_copy(vaug[1][:, c, 0:DV], vp[:, DV:P])

                for b in range(NSQB):
                    bsl = slice(b * SQB, (b + 1) * SQB)
                    a1t = a1t_pool.tile([P, SQB], BF16, tag="a1t")
                    at_ps = [
                        av_ps.tile([P, SQB], F32, name=f"at_ps{h}", tag="av")
                        for h in range(HPC)
                    ]
                    for g in range(NCH // 2):
                        if b == 0:
                            emit_v_chunk(2 * g)
                            emit_v_chunk(2 * g + 1)
                        pss = [
                            sc_ps.tile([P, 2 * SQB], F32, name=f"scps{h}", tag="sc")
                            for h in range(HPC)
                        ]
                        for j in range(2):
                            c = 2 * g + j
                            for h in range(HPC):
                                nc.tensor.matmul(
                                    pss[h][:, j * SQB : (j + 1) * SQB],
                                    lhsT=kpad[h][:, c * P : (c + 1) * P],
                                    rhs=qt[:, bsl],
                                    start=True, stop=True,
                                )
                        ess = []
                        for h in range(HPC):
                            es = est_pool.tile(
                                [P, 2 * SQB], BF16, name=f"est{h}", tag="est"
                            )
                            nc.scalar.activation(
                                es[:], pss[h][:], EXP, scale=float(SCALE)
                            )
                            ess.append(es)
                        for j in range(2):
                            c = 2 * g + j
                            for h in range(HPC):
                                nc.tensor.matmul(
                                    at_ps[h][0 : DV + 2, :],
                                    lhsT=vaug[h][:, c, :],
                                    rhs=ess[h][:, j * SQB : (j + 1) * SQB],
                                    start=(c == 0), stop=(c == NCH - 1),
                                )
                    # normalize: A1T rows = A^T * (1/rowsum) broadcast
                    for h in range(HPC):
                        rs0 = small.tile([1, SQB], F32, tag="rs0")
                        nc.vector.tensor_copy(rs0[:], at_ps[h][DV : DV + 1, :])
                        rsr = small.tile([1, SQB], F32, tag="rsr")
                        nc.vector.reciprocal_approx_fast(rsr[:], rs0[:])
                        bc = small.tile([P, SQB], F32, tag="bc")
                        nc.gpsimd.partition_broadcast(bc[0:DV, :], rsr[:])
                        if h == 0:
                            nc.vector.tensor_tensor(
                                a1t[0:DV, :], at_ps[h][0:DV, :], bc[0:DV, :], MULT
                            )
                        else:
                            tb = small.tile([P, SQB], BF16, tag="tb")
                            nc.vector.tensor_tensor(
                                tb[0:DV, :], at_ps[h][0:DV, :], bc[0:DV, :], MULT
                            )
                            nc.gpsimd.dma_start(a1t[DV:P, :], tb[0:DV, :])

                    # output projection for this block
                    for j in range(NSQB):
                        rsl = slice(b * SQB + j * P, b * SQB + (j + 1) * P)
                        osb = outp.tile([P, E], BF16, tag="osb")
                        for e2 in range(E // SQB):
                            esl = slice(e2 * SQB, (e2 + 1) * SQB)
                            if b == NSQB - 1:
                                ops = sc_ps.tile(
                                    [P, SQB], F32, name="ops2", tag="sc"
                                )
                            else:
                                ops = op_ps.tile(
                                    [P, SQB], F32, name="ops", tag="op"
                                )
                            nc.tensor.matmul(
                                ops[:],
                                lhsT=a1t[:, j * P : (j + 1) * P],
                                rhs=wosb[:, esl],
                                start=True, stop=True,
                            )
                            nc.vector.tensor_copy(osb[:, esl], ops[:])
                        nc.sync.dma_start(y_ap[rsl, :], osb[:])

    nc.compile()
    return nc


def kernel(x, Wq, Wk, Wv, Wo):
    global last_results
    x = np.asarray(x, dtype=np.float32)
    Wq = np.asarray(Wq, dtype=np.float32)
    Wk = np.asarray(Wk, dtype=np.float32)
    Wv = np.asarray(Wv, dtype=np.float32)
    Wo = np.asarray(Wo, dtype=np.float32)

    if "nc" not in _cache:
        _cache["nc"] = _build_nc()
    nc = _cache["nc"]

    bf = ml_dtypes.bfloat16
    # [E, S] -> [P, NE, S] partition-major (chunk n, partition p = row n*P+p)
    xT = np.ascontiguousarray(
        x.T.reshape(NE, P, S).transpose(1, 0, 2).astype(bf)
    )
    WqT = np.ascontiguousarray(Wq.T)
    WkT = np.ascontiguousarray(Wk.T)
    WvT = np.ascontiguousarray(Wv.T)
    WoT = np.ascontiguousarray(Wo.T)

    in_maps = []
    for i in range(NCORES):
        sl = slice(i * CSL, (i + 1) * CSL)

        def wslice(WT):
            # [E, CSL] slice -> [P, NE, CSL] partition-major
            return np.ascontiguousarray(
                WT[:, sl].reshape(NE, P, CSL).transpose(1, 0, 2).astype(bf)
            )

        in_maps.append({
            "xT": xT,
            "wqT": wslice(WqT),
            "wkT": wslice(WkT),
            "wvT": wslice(WvT),
            "woT": np.ascontiguousarray(WoT[sl, :].astype(bf)),
        })

    last_results = run_bass_kernel_spmd(
        nc, in_maps, core_ids=list(range(NCORES)), trace=TRACE
    )
    out = np.zeros((S, E), dtype=np.float32)
    for r in last_results.results:
        out += r["y"].astype(np.float32)
    return out
